# revision 23
# baseline (speedup 1.0000x reference)
"""Causal multi-head self-attention (B=2, S=2048, D=1024, H=16) on 8 trn2 cores.

Sharding: tensor-parallel over heads. Each core owns 2 heads (a 128-wide
slice of the QKV output dim / o_proj input dim), computes QKV projection,
causal attention and its partial output projection; the host sums the 8
partial outputs (the TP unshard step).

Device kernel layout (per core, SPMD):
  - Everything is computed in "transposed" orientation so no transposes of
    attention probabilities are ever needed:
      qT, kT  : [dloc=128, T]   (feature-major)  qT pre-scaled by 1/sqrt(dk)
      scoresT : [k_chunk=128, q_tile=512] = kT_chunk.T-contract @ qT
      PT      : exp(scoresT)  (block-causal; diagonal 128x128 blocks masked
                by a DVE multiply with an uploaded 0/1 mask)
      oT      : accum over k chunks of V_aug.T @ PT, where V_aug carries a
                64-wide ones block so PSUM rows 64:128 hold 64 replicated
                copies of the softmax denominator (free PE broadcast).
      out     : [tok, 1024] partial = oT(normalized) as lhsT @ woT
  - Scheduling: exp is batched over chunk pairs (one 2-bank PSUM tile per
    pair); heads are emitted interleaved; normalization runs directly from
    the attention PSUM accumulator (reciprocal_approx_fast + one multiply),
    and output-projection tiles trail the attention wavefront by two
    q-tiles.  The o_proj PSUM drain alternates between the Vector and
    Scalar engines to balance their queues.

dtype modes: "bf16" (default/fastest, rel err ~4e-3 vs the 2e-2 gate),
"f32r" (tf32-ish matmuls, ~2e-4), "f32" (exact, PE at 1/4 rate).
"""

import numpy as np
import ml_dtypes

import concourse.bass as bass
import concourse.mybir as mybir
import concourse.tile as tile
from concourse import bacc
from concourse.bass_utils import run_bass_kernel_spmd

# Problem config (hardcoded; harness contract).
B, S, D, NH = 2, 2048, 1024, 16
NCORES = 8
DK = D // NH                # 64
H_LOC = NH // NCORES        # 2 heads per core
DLOC = H_LOC * DK           # 128
T = B * S                   # 4096

MM_MODE = "bf16"            # "f32" | "f32r" | "bf16"

F32 = mybir.dt.float32
BF16 = mybir.dt.bfloat16
F32R = mybir.dt.float32r


def build_program(mm_mode=MM_MODE, b=B, s=S, d=D, nh=NH, ncores=NCORES):
    dk = d // nh
    h_loc = nh // ncores
    dloc = h_loc * dk
    t_all = b * s

    QT_ = 512                     # q tile size (matmul moving free dim)
    KC = 128                      # k chunk size (partition dim)
    assert s % QT_ == 0 and d % 128 == 0 and dloc == 128 and dk == 64

    n_qt = s // QT_               # q tiles per batch
    n_groups = t_all // QT_       # token groups for QKV projection
    n_kd = d // 128               # contraction chunks over d_model
    n_tc = t_all // KC            # token chunks (for V layout)

    # storage dtype of matmul operands ("f32r" must be materialized as
    # float32r end-to-end: walrus requires fp32r matmul inputs to be
    # *produced* rounded, so a bitcast at the matmul is rejected).
    st = {"bf16": BF16, "f32r": F32R, "f32": F32}[mm_mode]
    # dtype of the V transpose path (bf16 transposes run 2x on the PE and
    # halve the PSUM->SBUF copy cost of the per-chunk V tiles).
    vtdt = BF16 if mm_mode == "bf16" else F32
    # dtype of the partial output (bf16 halves the output HBM traffic; the
    # host accumulates the 8 partials in float64 anyway).
    odt = BF16 if mm_mode == "bf16" else F32

    nc = bacc.Bacc("TRN2", target_bir_lowering=False, debug=False,
                   enable_asserts=False)

    # xT / wqkvT are host-pre-swizzled into the exact SBUF tile layouts so
    # every DMA is fully contiguous (strided patterns measured ~4x slower)
    xT = nc.dram_tensor("xT", [n_groups, 128, n_kd, QT_], st,
                        kind="ExternalInput")
    wqkvT = nc.dram_tensor("wqkvT", [128, n_kd, 3 * dloc], st,
                           kind="ExternalInput")
    woT = nc.dram_tensor("woT", [dloc, d], st, kind="ExternalInput")
    dmask = nc.dram_tensor("dmask", [128, 128], st, kind="ExternalInput")
    ident = nc.dram_tensor("ident", [128, 128], vtdt, kind="ExternalInput")
    part = nc.dram_tensor("part", [t_all, d], odt, kind="ExternalOutput")

    with tile.TileContext(nc) as tc:
        with (
            tc.tile_pool(name="persist", bufs=1) as pp,
            tc.tile_pool(name="small", bufs=4) as sp,
        ):
            # ---- persistent SBUF tensors ----
            # (wo/dmask/ident DMAs are emitted after the wq DMA inside
            # phase 1: the gpsimd SWDGE queue is FIFO and wq gates the
            # first QKV matmul, while these are needed much later)
            wo_sb = pp.tile([dloc, d], st, tag="wo")
            dm_sb = pp.tile([128, 128], st, tag="dm")
            id_sb = pp.tile([128, 128], vtdt, tag="id")

            # scratch for PE warm-up matmuls (HAM clock-gate release): the
            # first ~3.4us of matmul activity run at 1.2 GHz; burn that
            # window on dummy matmuls while the first input DMAs land.
            warm_sb = pp.tile([128, 512], st, tag="warm")
            nc.vector.memset(warm_sb[:], 0.0)

            qT_sb = pp.tile([128, t_all], st, tag="qT")
            kT_sb = pp.tile([128, t_all], st, tag="kT")
            # V_aug per head: [128 tok, n_chunks, 128]; cols 64:128 = 1.0 so
            # the AV matmul lands 64 replicated softmax-denominator rows in
            # PSUM (free PE broadcast).
            va_sb = [pp.tile([128, n_tc, 128], st, tag=f"va{h}",
                             name=f"va{h}") for h in range(h_loc)]
            ones_c = pp.tile([128, 1], F32, tag="ones_c")
            nc.vector.memset(ones_c[:], 1.0)
            for h in range(h_loc):
                nc.vector.tensor_copy(
                    va_sb[h][:, :, dk:128],
                    ones_c[:, :, None].broadcast_to((128, n_tc, 128 - dk)))
            oT_sb = pp.tile([128, t_all], st, tag="oT")

            # ---- phase 1: QKV projection (+ V transpose) ----
            with (
                tc.tile_pool(name="qkvw", bufs=1) as wp,
                tc.tile_pool(name="xg", bufs=2) as xp,
                tc.tile_pool(name="vtmp", bufs=1) as vp,
                tc.tile_pool(name="ps_qkv", bufs=3, space="PSUM") as pqk,
                tc.tile_pool(name="ps_tr", bufs=2, space="PSUM") as ptr,
            ):
                # PE warm-up: dummy matmuls with no DMA dependency bridge
                # the ~10us initial input-DMA wait (HBM is shared by all 8
                # SPMD cores loading at once) and keep the HAM clock gate at
                # 8/8 so QKV starts at 2.4 GHz.
                for w in range(20):
                    pw = pqk.tile([128, QT_], F32, tag="ps")
                    nc.tensor.matmul(pw[:], warm_sb[:, 0:128], warm_sb[:],
                                     start=True, stop=True)

                wq_sb = wp.tile([128, n_kd, 3 * dloc], st, tag="wq")
                # wq on sync (HWDGE, contiguous); first chunks in their own
                # smaller DMA so the QKV accumulation can start sooner
                nc.sync.dma_start(out=wq_sb[:, 0:2, :],
                                  in_=wqkvT.ap()[:, 0:2, :])
                nc.sync.dma_start(out=wq_sb[:, 2:n_kd, :],
                                  in_=wqkvT.ap()[:, 2:n_kd, :])
                nc.gpsimd.dma_start(out=id_sb[:], in_=ident[:, :])
                nc.gpsimd.dma_start(out=dm_sb[:], in_=dmask[:, :])
                nc.gpsimd.dma_start(out=wo_sb[:], in_=woT[:, :])
                vT_tmp = vp.tile([128, t_all], vtdt, tag="vtmp")

                for g in range(n_groups):
                    xg = xp.tile([128, n_kd, QT_], st, tag="xg")
                    # contiguous group DMA on the scalar HWDGE queue
                    # (parallel with wq on sync); group 0's first chunks
                    # split out so the first matmul starts sooner
                    if g == 0:
                        nc.scalar.dma_start(out=xg[:, 0:2, :],
                                            in_=xT.ap()[g, :, 0:2, :])
                        nc.scalar.dma_start(out=xg[:, 2:n_kd, :],
                                            in_=xT.ap()[g, :, 2:n_kd, :])
                    else:
                        nc.scalar.dma_start(out=xg[:], in_=xT.ap()[g])
                    for m in range(3):  # 0:Q 1:K 2:V
                        ps = pqk.tile([128, QT_], F32, tag="ps")
                        for kd in range(n_kd):
                            nc.tensor.matmul(
                                ps[:],
                                wq_sb[:, kd, m * 128:(m + 1) * 128],
                                xg[:, kd, :],
                                start=(kd == 0), stop=(kd == n_kd - 1),
                            )
                        gsl = slice(g * QT_, (g + 1) * QT_)
                        if m == 0:
                            nc.vector.tensor_scalar_mul(
                                qT_sb[:, gsl], ps[:], 1.0 / float(np.sqrt(dk)))
                        elif m == 1:
                            nc.vector.tensor_copy(kT_sb[:, gsl], ps[:])
                        else:
                            nc.vector.tensor_copy(vT_tmp[:, gsl], ps[:])
                    # V transpose for this group's 4 chunks, interleaved into
                    # the QKV matmul stream (PE has headroom here; doing all
                    # 32 transposes after the last group stalls attention)
                    for ci in range(g * (QT_ // 128), (g + 1) * (QT_ // 128)):
                        pst = ptr.tile([128, 128], vtdt, tag="pst")
                        nc.tensor.transpose(
                            pst[:], vT_tmp[:, ci * 128:(ci + 1) * 128],
                            id_sb[:])
                        for h in range(h_loc):
                            nc.vector.tensor_copy(
                                va_sb[h][:, ci, 0:dk],
                                pst[:, h * dk:(h + 1) * dk])

            # ---- phase 2: attention + output projection ----
            # Chunk PAIRS share one 2-bank PSUM tile and one exp ACTIVATE
            # (halves ScalarE per-op overhead); heads are emitted
            # interleaved so the PE always has an independent chain.
            # Normalization reads the po accumulator in place (approx
            # reciprocal + one multiply); output-projection tiles trail the
            # attention wavefront by two q-tiles.
            with (
                tc.tile_pool(name="pt", bufs=6) as ptp,
                tc.tile_pool(name="onb", bufs=4) as onp,
                tc.tile_pool(name="rbp", bufs=3) as rbp,
                tc.tile_pool(name="ob", bufs=3) as obp,
                tc.tile_pool(name="ps_s", bufs=2, space="PSUM") as pss,
                tc.tile_pool(name="ps_o", bufs=1, space="PSUM") as pso,
                tc.tile_pool(name="ps_x", bufs=2, space="PSUM") as psx,
            ):
                def emit_normalize(h, onb_n, onb_d, q0_):
                    hp_ = slice(h * dk, (h + 1) * dk)
                    rb = rbp.tile([dk, QT_], F32, tag="rb", name="rb")
                    nc.vector.reciprocal_approx_fast(rb[:], onb_d[:])
                    nc.vector.tensor_mul(
                        oT_sb[hp_, q0_:q0_ + QT_], onb_n[:], rb[:])

                def emit_odma(tg, ob, on_scalar=False):
                    # output DMAs stay on sync mid-run (DIRECT2D descriptor
                    # generation on the scalar engine would steal ~0.7us per
                    # DMA from the exp stream); the tail may use scalar.
                    eng = nc.scalar if on_scalar else nc.sync
                    eng.dma_start(out=part[tg:tg + 128, :], in_=ob[:])

                def emit_outproj(bi_, qt_, drain="v"):
                    for ti, tt in enumerate(range(4 * qt_, 4 * qt_ + 4)):
                        tg = bi_ * s + tt * 128
                        ob = obp.tile([128, d], odt, tag="ob", name="ob")
                        use_s = drain == "s" or (drain == "alt" and ti % 2)
                        for no in range(d // 512):
                            px = psx.tile([128, 512], F32, tag="px",
                                          name="px")
                            nc.tensor.matmul(
                                px[:],
                                oT_sb[:, tg:tg + 128],
                                wo_sb[:, no * 512:(no + 1) * 512],
                                start=True, stop=True,
                            )
                            if use_s:
                                nc.scalar.copy(
                                    ob[:, no * 512:(no + 1) * 512], px[:])
                            else:
                                nc.vector.tensor_copy(
                                    ob[:, no * 512:(no + 1) * 512], px[:])
                        emit_odma(tg, ob)

                out_queue = []
                pending = []
                for bi in range(b):
                    for qt in range(n_qt):
                        q0 = bi * s + qt * QT_      # global q start
                        n_full = qt * (QT_ // KC)   # full k chunks
                        n_chunks = n_full + (QT_ // KC)
                        po = [pso.tile([128, QT_], F32, tag=f"po{h}",
                                       name=f"po{h}") for h in range(h_loc)]
                        for pair0 in range(0, n_chunks, 2):
                            for h in range(h_loc):
                                hp = slice(h * dk, (h + 1) * dk)
                                ps = pss.tile([128, 2, QT_], F32, tag="ps")
                                qoffs = []
                                for jj in range(2):
                                    kc = pair0 + jj
                                    masked = kc >= n_full
                                    j = kc - n_full if masked else 0
                                    qoff = 128 * j if masked else 0
                                    qoffs.append(qoff)
                                    k0 = bi * s + kc * KC
                                    nc.tensor.matmul(
                                        ps[:, jj, qoff:QT_],
                                        kT_sb[hp, k0:k0 + KC],
                                        qT_sb[hp, q0 + qoff:q0 + QT_],
                                        start=True, stop=True,
                                        skip_group_check=True,
                                    )
                                pt = ptp.tile([128, 2, QT_], st, tag="pt")
                                if qoffs[0] == qoffs[1]:
                                    nc.scalar.activation(
                                        pt[:, :, qoffs[0]:QT_],
                                        ps[:, :, qoffs[0]:QT_],
                                        mybir.ActivationFunctionType.Exp)
                                else:
                                    for jj in range(2):
                                        nc.scalar.activation(
                                            pt[:, jj, qoffs[jj]:QT_],
                                            ps[:, jj, qoffs[jj]:QT_],
                                            mybir.ActivationFunctionType.Exp)
                                for jj in range(2):
                                    kc = pair0 + jj
                                    qoff = qoffs[jj]
                                    if kc >= n_full:
                                        nc.vector.tensor_mul(
                                            pt[:, jj, qoff:qoff + 128],
                                            pt[:, jj, qoff:qoff + 128],
                                            dm_sb[:])
                                    nc.tensor.matmul(
                                        po[h][:, qoff:QT_],
                                        va_sb[h][:, bi * (s // KC) + kc, :],
                                        pt[:, jj, qoff:QT_],
                                        start=(kc == 0),
                                        stop=(kc == n_chunks - 1),
                                        skip_group_check=True,
                                    )
                        onb = []
                        for h in range(h_loc):
                            # DVE copies free the po bank (bufs=1) fast
                            o_n = onp.tile([dk, QT_], F32, tag=f"onbn{h}",
                                           name=f"onbn{h}")
                            nc.vector.tensor_copy(o_n[:], po[h][0:dk, :])
                            o_d = onp.tile([dk, QT_], F32, tag=f"onbd{h}",
                                           name=f"onbd{h}")
                            nc.vector.tensor_copy(o_d[:], po[h][dk:2 * dk, :])
                            onb.append((o_n, o_d))
                        if len(out_queue) >= 2:
                            emit_outproj(*out_queue.pop(0))
                        for item in pending:
                            emit_normalize(*item)
                        pending = [(h, onb[h][0], onb[h][1], q0)
                                   for h in range(h_loc)]
                        out_queue.append((bi, qt))
                # tail: the ACT engine has no exp work left, so drain the
                # next-to-last tile's o_proj there while DVE runs the final
                # normalize.  The last tile normalizes per 128-token slab so
                # its o_proj matmuls start before the whole tile is done;
                # drains alternate DVE/ACT.
                emit_outproj(*out_queue.pop(0), drain="s")
                rbs = []
                for h, o_n, o_d, q0_ in pending:
                    rb = rbp.tile([dk, QT_], F32, tag="rb", name="rb")
                    nc.vector.reciprocal_approx_fast(rb[:], o_d[:])
                    rbs.append((h, o_n, rb, q0_))
                bi_, qt_ = out_queue.pop(0)
                for ti in range(4):
                    sl = slice(ti * 128, (ti + 1) * 128)
                    for h, o_n, rb, q0_ in rbs:
                        hp_ = slice(h * dk, (h + 1) * dk)
                        nc.vector.tensor_mul(
                            oT_sb[hp_, q0_ + ti * 128:q0_ + (ti + 1) * 128],
                            o_n[:, sl], rb[:, sl])
                    tt = 4 * qt_ + ti
                    tg = bi_ * s + tt * 128
                    ob = obp.tile([128, d], odt, tag="ob", name="ob")
                    for no in range(d // 512):
                        px = psx.tile([128, 512], F32, tag="px", name="px")
                        nc.tensor.matmul(
                            px[:],
                            oT_sb[:, tg:tg + 128],
                            wo_sb[:, no * 512:(no + 1) * 512],
                            start=True, stop=True,
                        )
                        if ti % 2:
                            nc.scalar.copy(
                                ob[:, no * 512:(no + 1) * 512], px[:])
                        else:
                            nc.vector.tensor_copy(
                                ob[:, no * 512:(no + 1) * 512], px[:])
                    emit_odma(tg, ob, on_scalar=(ti % 2 == 0))

    nc.compile()
    return nc


_NC_CACHE = {}


def _get_program(mm_mode=None):
    if mm_mode is None:
        mm_mode = MM_MODE
    if mm_mode not in _NC_CACHE:
        _NC_CACHE[mm_mode] = build_program(mm_mode)
    return _NC_CACHE[mm_mode]


def make_host_inputs(in_features, qkv_proj_weight, o_proj_weight,
                     mm_mode=None, ncores=NCORES):
    """Build the per-core input maps (the TP shard step)."""
    x = np.asarray(in_features, dtype=np.float32)
    qkv = np.asarray(qkv_proj_weight, dtype=np.float32)
    wo = np.asarray(o_proj_weight, dtype=np.float32)
    b, s, d = x.shape
    if mm_mode is None:
        mm_mode = MM_MODE
    np_st = ml_dtypes.bfloat16 if mm_mode == "bf16" else np.float32
    np_vt = ml_dtypes.bfloat16 if mm_mode == "bf16" else np.float32

    # pre-swizzled fully-contiguous DMA layouts (see build_program):
    # xT[g, p, c, n] = x[g*512+n, c*128+p]
    qt_, t_all = 512, b * s
    n_groups, n_kd = t_all // qt_, d // 128
    xTg = np.ascontiguousarray(
        x.reshape(n_groups, qt_, n_kd, 128).transpose(0, 3, 2, 1)
    ).astype(np_st)
    kk = np.arange(128)[:, None]
    qq = np.arange(128)[None, :]
    dmask = (qq >= kk).astype(np_st)
    ident = np.eye(128, dtype=np_vt)

    in_maps = []
    for c in range(ncores):
        rows = slice(c * DLOC, (c + 1) * DLOC)
        wstack = np.concatenate([qkv[i][rows, :] for i in range(3)], axis=0)
        # wqkvT[p, c, n] = wstack[n, c*128+p]
        wqkvT = np.ascontiguousarray(
            wstack.T.reshape(n_kd, 128, 3 * DLOC).transpose(1, 0, 2)
        ).astype(np_st)
        woT = np.ascontiguousarray(wo[:, rows].T).astype(np_st)
        in_maps.append({"xT": xTg, "wqkvT": wqkvT, "woT": woT,
                        "dmask": dmask, "ident": ident})
    return in_maps


def kernel(**inputs):
    nh = inputs.get("num_heads", NH)
    nh = int(np.asarray(nh)) if not isinstance(nh, int) else nh
    assert nh == NH, f"kernel hardcoded for {NH} heads, got {nh}"

    nc = _get_program()
    in_maps = make_host_inputs(inputs["in_features"],
                               inputs["qkv_proj_weight"],
                               inputs["o_proj_weight"])
    res = run_bass_kernel_spmd(nc, in_maps, list(range(NCORES)))
    acc = np.zeros((T, D), dtype=np.float64)
    for c in range(NCORES):
        acc += np.asarray(res.results[c]["part"], dtype=np.float64)
    return acc.reshape(B, S, D).astype(np.float32)


# revision 32
# speedup vs baseline: 1.0475x; 1.0475x over previous
"""Causal multi-head self-attention (B=2, S=2048, D=1024, H=16) on 8 trn2 cores.

Sharding: tensor-parallel over heads. Each core owns 2 heads (a 128-wide
slice of the QKV output dim / o_proj input dim), computes QKV projection,
causal attention and its partial output projection; the host sums the 8
partial outputs (the TP unshard step).

Device kernel layout (per core, SPMD):
  - Everything is computed in "transposed" orientation so no transposes of
    attention probabilities are ever needed:
      qT, kT  : [dloc=128, T]   (feature-major)  qT pre-scaled by 1/sqrt(dk)
      scoresT : [k_chunk=128, q_tile=512] = kT_chunk.T-contract @ qT
      PT      : exp(scoresT)  (block-causal; diagonal 128x128 blocks masked
                by a DVE multiply with an uploaded 0/1 mask)
      oT      : accum over k chunks of V_aug.T @ PT, where V_aug carries a
                64-wide ones block so PSUM rows 64:128 hold 64 replicated
                copies of the softmax denominator (free PE broadcast).
      out     : [tok, 1024] partial = oT(normalized) as lhsT @ woT
  - Scheduling: exp is batched over chunk pairs (one 2-bank PSUM tile per
    pair); heads are emitted interleaved; normalization runs directly from
    the attention PSUM accumulator (reciprocal_approx_fast + one multiply),
    and output-projection tiles trail the attention wavefront by two
    q-tiles.  The o_proj PSUM drain alternates between the Vector and
    Scalar engines to balance their queues.

dtype modes: "bf16" (default/fastest, rel err ~4e-3 vs the 2e-2 gate),
"f32r" (tf32-ish matmuls, ~2e-4), "f32" (exact, PE at 1/4 rate).
"""

import numpy as np
import ml_dtypes

import concourse.bass as bass
import concourse.mybir as mybir
import concourse.tile as tile
from concourse import bacc
from concourse.bass_utils import run_bass_kernel_spmd

# Problem config (hardcoded; harness contract).
B, S, D, NH = 2, 2048, 1024, 16
NCORES = 8
DK = D // NH                # 64
H_LOC = NH // NCORES        # 2 heads per core
DLOC = H_LOC * DK           # 128
T = B * S                   # 4096

MM_MODE = "bf16"            # "f32" | "f32r" | "bf16"

F32 = mybir.dt.float32
BF16 = mybir.dt.bfloat16
F32R = mybir.dt.float32r


def build_program(mm_mode=MM_MODE, b=B, s=S, d=D, nh=NH, ncores=NCORES):
    dk = d // nh
    h_loc = nh // ncores
    dloc = h_loc * dk
    t_all = b * s

    QT_ = 512                     # q tile size (matmul moving free dim)
    KC = 128                      # k chunk size (partition dim)
    assert s % QT_ == 0 and d % 128 == 0 and dloc == 128 and dk == 64

    n_qt = s // QT_               # q tiles per batch
    n_groups = t_all // QT_       # token groups for QKV projection
    n_kd = d // 128               # contraction chunks over d_model
    n_tc = t_all // KC            # token chunks (for V layout)

    # storage dtype of matmul operands ("f32r" must be materialized as
    # float32r end-to-end: walrus requires fp32r matmul inputs to be
    # *produced* rounded, so a bitcast at the matmul is rejected).
    st = {"bf16": BF16, "f32r": F32R, "f32": F32}[mm_mode]
    # P/V dtype for the *unmasked* AV matmuls: fp8e4m3 enables DoubleRow
    # (2 k-chunks per pass, ~1.8x AV throughput).  exp(s-1) lands in
    # [e^-7, ~150] which e4m3 covers; the shift and the quantization cancel
    # in the softmax normalization (the ones-block denominator sums the
    # same quantized P).  Masked (diagonal) chunks stay bf16: short causal
    # rows average over few keys, so fp8 noise would not cancel there.
    avdt = mybir.dt.float8e4 if mm_mode == "bf16" else st
    av_fp8 = avdt == mybir.dt.float8e4
    exp_bias = -1.0 if av_fp8 else 0.0
    # dtype of the V transpose path (bf16 transposes run 2x on the PE and
    # halve the PSUM->SBUF copy cost of the per-chunk V tiles).
    vtdt = BF16 if mm_mode == "bf16" else F32
    # dtype of the partial output (bf16 halves the output HBM traffic; the
    # host accumulates the 8 partials in float64 anyway).
    odt = BF16 if mm_mode == "bf16" else F32

    nc = bacc.Bacc("TRN2", target_bir_lowering=False, debug=False,
                   enable_asserts=False)

    # xT / wqkvT are host-pre-swizzled into the exact SBUF tile layouts so
    # every DMA is fully contiguous (strided patterns measured ~4x slower)
    xT = nc.dram_tensor("xT", [n_groups, 128, n_kd, QT_], st,
                        kind="ExternalInput")
    wqkvT = nc.dram_tensor("wqkvT", [128, n_kd, 3 * dloc], st,
                           kind="ExternalInput")
    woT = nc.dram_tensor("woT", [dloc, d], st, kind="ExternalInput")
    dmask = nc.dram_tensor("dmask", [128, 128], st, kind="ExternalInput")
    ident = nc.dram_tensor("ident", [128, 128], vtdt, kind="ExternalInput")
    part = nc.dram_tensor("part", [t_all, d], odt, kind="ExternalOutput")

    with tile.TileContext(nc) as tc:
        with (
            tc.tile_pool(name="persist", bufs=1) as pp,
            tc.tile_pool(name="small", bufs=4) as sp,
        ):
            # ---- persistent SBUF tensors ----
            # (wo/dmask/ident DMAs are emitted after the wq DMA inside
            # phase 1: the gpsimd SWDGE queue is FIFO and wq gates the
            # first QKV matmul, while these are needed much later)
            wo_sb = pp.tile([dloc, d], st, tag="wo")
            dm_sb = pp.tile([128, 128], st, tag="dm")
            id_sb = pp.tile([128, 128], vtdt, tag="id")

            # scratch for PE warm-up matmuls (HAM clock-gate release): the
            # first ~3.4us of matmul activity run at 1.2 GHz; burn that
            # window on dummy matmuls while the first input DMAs land.
            warm_sb = pp.tile([128, 512], st, tag="warm")
            nc.vector.memset(warm_sb[:], 0.0)

            qT_sb = pp.tile([128, t_all], st, tag="qT")
            kT_sb = pp.tile([128, t_all], st, tag="kT")
            # V_aug per head: [128 tok, n_chunks, 128]; cols 64:128 = 1.0 so
            # the AV matmul lands 64 replicated softmax-denominator rows in
            # PSUM (free PE broadcast).
            va_sb = [pp.tile([128, n_tc, 128], st, tag=f"va{h}",
                             name=f"va{h}") for h in range(h_loc)]
            va8_sb = [pp.tile([128, n_tc, 128], avdt, tag=f"va8{h}",
                              name=f"va8{h}") for h in range(h_loc)
                      ] if av_fp8 else None
            ones_c = pp.tile([128, 1], F32, tag="ones_c")
            nc.vector.memset(ones_c[:], 1.0)
            ebias_c = pp.tile([128, 1], F32, tag="ebias_c")
            nc.vector.memset(ebias_c[:], exp_bias)
            for h in range(h_loc):
                nc.vector.tensor_copy(
                    va_sb[h][:, :, dk:128],
                    ones_c[:, :, None].broadcast_to((128, n_tc, 128 - dk)))
                if av_fp8:
                    nc.vector.tensor_copy(
                        va8_sb[h][:, :, dk:128],
                        ones_c[:, :, None].broadcast_to(
                            (128, n_tc, 128 - dk)))
            oT_sb = pp.tile([128, t_all], st, tag="oT")

            # ---- phase 1: QKV projection (+ V transpose) ----
            with (
                tc.tile_pool(name="qkvw", bufs=1) as wp,
                tc.tile_pool(name="xg", bufs=2) as xp,
                tc.tile_pool(name="vtmp", bufs=1) as vp,
                tc.tile_pool(name="ps_qkv", bufs=3, space="PSUM") as pqk,
                tc.tile_pool(name="ps_tr", bufs=2, space="PSUM") as ptr,
            ):
                # PE warm-up: dummy matmuls with no DMA dependency bridge
                # the ~10us initial input-DMA wait (HBM is shared by all 8
                # SPMD cores loading at once) and keep the HAM clock gate at
                # 8/8 so QKV starts at 2.4 GHz.
                for w in range(20):
                    pw = pqk.tile([128, QT_], F32, tag="ps")
                    nc.tensor.matmul(pw[:], warm_sb[:, 0:128], warm_sb[:],
                                     start=True, stop=True)

                wq_sb = wp.tile([128, n_kd, 3 * dloc], st, tag="wq")
                # wq on sync (HWDGE, contiguous); first chunks in their own
                # smaller DMA so the QKV accumulation can start sooner
                nc.sync.dma_start(out=wq_sb[:, 0:2, :],
                                  in_=wqkvT.ap()[:, 0:2, :])
                nc.sync.dma_start(out=wq_sb[:, 2:n_kd, :],
                                  in_=wqkvT.ap()[:, 2:n_kd, :])
                nc.gpsimd.dma_start(out=id_sb[:], in_=ident[:, :])
                nc.gpsimd.dma_start(out=dm_sb[:], in_=dmask[:, :])
                nc.gpsimd.dma_start(out=wo_sb[:], in_=woT[:, :])
                vT_tmp = vp.tile([128, t_all], vtdt, tag="vtmp")

                for g in range(n_groups):
                    xg = xp.tile([128, n_kd, QT_], st, tag="xg")
                    # contiguous group DMA on the scalar HWDGE queue
                    # (parallel with wq on sync); group 0's first chunks
                    # split out so the first matmul starts sooner
                    if g == 0:
                        nc.scalar.dma_start(out=xg[:, 0:2, :],
                                            in_=xT.ap()[g, :, 0:2, :])
                        nc.scalar.dma_start(out=xg[:, 2:n_kd, :],
                                            in_=xT.ap()[g, :, 2:n_kd, :])
                    else:
                        nc.scalar.dma_start(out=xg[:], in_=xT.ap()[g])
                    for m in range(3):  # 0:Q 1:K 2:V
                        ps = pqk.tile([128, QT_], F32, tag="ps")
                        for kd in range(n_kd):
                            nc.tensor.matmul(
                                ps[:],
                                wq_sb[:, kd, m * 128:(m + 1) * 128],
                                xg[:, kd, :],
                                start=(kd == 0), stop=(kd == n_kd - 1),
                            )
                        gsl = slice(g * QT_, (g + 1) * QT_)
                        if m == 0:
                            nc.vector.tensor_scalar_mul(
                                qT_sb[:, gsl], ps[:], 1.0 / float(np.sqrt(dk)))
                        elif m == 1:
                            nc.vector.tensor_copy(kT_sb[:, gsl], ps[:])
                        else:
                            nc.vector.tensor_copy(vT_tmp[:, gsl], ps[:])
                    # V transpose for this group's 4 chunks, interleaved into
                    # the QKV matmul stream (PE has headroom here; doing all
                    # 32 transposes after the last group stalls attention)
                    for ci in range(g * (QT_ // 128), (g + 1) * (QT_ // 128)):
                        pst = ptr.tile([128, 128], vtdt, tag="pst")
                        nc.tensor.transpose(
                            pst[:], vT_tmp[:, ci * 128:(ci + 1) * 128],
                            id_sb[:])
                        for h in range(h_loc):
                            nc.vector.tensor_copy(
                                va_sb[h][:, ci, 0:dk],
                                pst[:, h * dk:(h + 1) * dk])
                            if av_fp8:
                                nc.vector.tensor_copy(
                                    va8_sb[h][:, ci, 0:dk],
                                    pst[:, h * dk:(h + 1) * dk])

            # ---- phase 2: attention + output projection ----
            # Chunk PAIRS share one 2-bank PSUM tile and one exp ACTIVATE
            # (halves ScalarE per-op overhead); heads are emitted
            # interleaved so the PE always has an independent chain.
            # Normalization reads the po accumulator in place (approx
            # reciprocal + one multiply); output-projection tiles trail the
            # attention wavefront by two q-tiles.
            with (
                tc.tile_pool(name="pt", bufs=6) as ptp,
                tc.tile_pool(name="onb", bufs=4) as onp,
                tc.tile_pool(name="rbp", bufs=3) as rbp,
                tc.tile_pool(name="ob", bufs=3) as obp,
                tc.tile_pool(name="ps_s", bufs=2, space="PSUM") as pss,
                tc.tile_pool(name="ps_o", bufs=1, space="PSUM") as pso,
                tc.tile_pool(name="ps_x", bufs=2, space="PSUM") as psx,
            ):
                def emit_normalize(h, onb_n, onb_d, q0_):
                    hp_ = slice(h * dk, (h + 1) * dk)
                    rb = rbp.tile([dk, QT_], F32, tag="rb", name="rb")
                    nc.vector.reciprocal_approx_fast(rb[:], onb_d[:])
                    nc.vector.tensor_mul(
                        oT_sb[hp_, q0_:q0_ + QT_], onb_n[:], rb[:])

                def emit_odma(tg, ob, on_scalar=False):
                    # output DMAs stay on sync mid-run (DIRECT2D descriptor
                    # generation on the scalar engine would steal ~0.7us per
                    # DMA from the exp stream); the tail may use scalar.
                    eng = nc.scalar if on_scalar else nc.sync
                    eng.dma_start(out=part[tg:tg + 128, :], in_=ob[:])

                def emit_outproj(bi_, qt_, drain="v"):
                    for ti, tt in enumerate(range(4 * qt_, 4 * qt_ + 4)):
                        tg = bi_ * s + tt * 128
                        ob = obp.tile([128, d], odt, tag="ob", name="ob")
                        use_s = drain == "s" or (drain == "alt" and ti % 2)
                        for no in range(d // 512):
                            px = psx.tile([128, 512], F32, tag="px",
                                          name="px")
                            nc.tensor.matmul(
                                px[:],
                                oT_sb[:, tg:tg + 128],
                                wo_sb[:, no * 512:(no + 1) * 512],
                                start=True, stop=True,
                            )
                            if use_s:
                                nc.scalar.copy(
                                    ob[:, no * 512:(no + 1) * 512], px[:])
                            else:
                                nc.vector.tensor_copy(
                                    ob[:, no * 512:(no + 1) * 512], px[:])
                        emit_odma(tg, ob)

                out_queue = []
                pending = []
                for bi in range(b):
                    for qt in range(n_qt):
                        q0 = bi * s + qt * QT_      # global q start
                        n_full = qt * (QT_ // KC)   # full k chunks
                        n_chunks = n_full + (QT_ // KC)
                        po = [pso.tile([128, QT_], F32, tag=f"po{h}",
                                       name=f"po{h}") for h in range(h_loc)]
                        for pair0 in range(0, n_chunks, 2):
                            for h in range(h_loc):
                                hp = slice(h * dk, (h + 1) * dk)
                                ps = pss.tile([128, 2, QT_], F32, tag="ps")
                                qoffs = []
                                for jj in range(2):
                                    kc = pair0 + jj
                                    masked = kc >= n_full
                                    j = kc - n_full if masked else 0
                                    qoff = 128 * j if masked else 0
                                    qoffs.append(qoff)
                                    k0 = bi * s + kc * KC
                                    nc.tensor.matmul(
                                        ps[:, jj, qoff:QT_],
                                        kT_sb[hp, k0:k0 + KC],
                                        qT_sb[hp, q0 + qoff:q0 + QT_],
                                        start=True, stop=True,
                                        skip_group_check=True,
                                    )
                                full_pair = av_fp8 and pair0 + 1 < n_full
                                pt = ptp.tile([128, 2, QT_],
                                              avdt if full_pair else st,
                                              tag="pt8" if full_pair
                                              else "pt")
                                if qoffs[0] == qoffs[1]:
                                    nc.scalar.activation(
                                        pt[:, :, qoffs[0]:QT_],
                                        ps[:, :, qoffs[0]:QT_],
                                        mybir.ActivationFunctionType.Exp,
                                        bias=ebias_c[:, 0:1])
                                else:
                                    for jj in range(2):
                                        nc.scalar.activation(
                                            pt[:, jj, qoffs[jj]:QT_],
                                            ps[:, jj, qoffs[jj]:QT_],
                                            mybir.ActivationFunctionType.Exp,
                                            bias=ebias_c[:, 0:1])
                                base = bi * (s // KC)
                                if full_pair:
                                    # unmasked pair: one DoubleRow matmul
                                    # consumes both k-chunks in 512 passes
                                    nc.tensor.matmul(
                                        po[h][:],
                                        va8_sb[h][:, base + pair0:
                                                  base + pair0 + 2, :],
                                        pt[:, :, :],
                                        start=(pair0 == 0),
                                        stop=False,
                                        perf_mode=(
                                            mybir.MatmulPerfMode.DoubleRow),
                                        skip_group_check=True,
                                    )
                                    continue
                                for jj in range(2):
                                    kc = pair0 + jj
                                    qoff = qoffs[jj]
                                    if kc >= n_full:
                                        nc.vector.tensor_mul(
                                            pt[:, jj, qoff:qoff + 128],
                                            pt[:, jj, qoff:qoff + 128],
                                            dm_sb[:])
                                    nc.tensor.matmul(
                                        po[h][:, qoff:QT_],
                                        va_sb[h][:, base + kc, :],
                                        pt[:, jj, qoff:QT_],
                                        start=(kc == 0),
                                        stop=(kc == n_chunks - 1),
                                        skip_group_check=True,
                                    )
                        onb = []
                        for h in range(h_loc):
                            # DVE copies free the po bank (bufs=1) fast
                            o_n = onp.tile([dk, QT_], F32, tag=f"onbn{h}",
                                           name=f"onbn{h}")
                            nc.vector.tensor_copy(o_n[:], po[h][0:dk, :])
                            o_d = onp.tile([dk, QT_], F32, tag=f"onbd{h}",
                                           name=f"onbd{h}")
                            nc.vector.tensor_copy(o_d[:], po[h][dk:2 * dk, :])
                            onb.append((o_n, o_d))
                        if len(out_queue) >= 2:
                            emit_outproj(*out_queue.pop(0))
                        for item in pending:
                            emit_normalize(*item)
                        pending = [(h, onb[h][0], onb[h][1], q0)
                                   for h in range(h_loc)]
                        out_queue.append((bi, qt))
                # tail: the ACT engine has no exp work left, so drain the
                # next-to-last tile's o_proj there while DVE runs the final
                # normalize.  The last tile normalizes per 128-token slab so
                # its o_proj matmuls start before the whole tile is done;
                # drains alternate DVE/ACT.
                emit_outproj(*out_queue.pop(0), drain="s")
                rbs = []
                for h, o_n, o_d, q0_ in pending:
                    rb = rbp.tile([dk, QT_], F32, tag="rb", name="rb")
                    nc.vector.reciprocal_approx_fast(rb[:], o_d[:])
                    rbs.append((h, o_n, rb, q0_))
                bi_, qt_ = out_queue.pop(0)
                for ti in range(4):
                    sl = slice(ti * 128, (ti + 1) * 128)
                    for h, o_n, rb, q0_ in rbs:
                        hp_ = slice(h * dk, (h + 1) * dk)
                        nc.vector.tensor_mul(
                            oT_sb[hp_, q0_ + ti * 128:q0_ + (ti + 1) * 128],
                            o_n[:, sl], rb[:, sl])
                    tt = 4 * qt_ + ti
                    tg = bi_ * s + tt * 128
                    ob = obp.tile([128, d], odt, tag="ob", name="ob")
                    for no in range(d // 512):
                        px = psx.tile([128, 512], F32, tag="px", name="px")
                        nc.tensor.matmul(
                            px[:],
                            oT_sb[:, tg:tg + 128],
                            wo_sb[:, no * 512:(no + 1) * 512],
                            start=True, stop=True,
                        )
                        if ti % 2:
                            nc.scalar.copy(
                                ob[:, no * 512:(no + 1) * 512], px[:])
                        else:
                            nc.vector.tensor_copy(
                                ob[:, no * 512:(no + 1) * 512], px[:])
                    emit_odma(tg, ob, on_scalar=(ti % 2 == 0))

    nc.compile()
    return nc


_NC_CACHE = {}


def _get_program(mm_mode=None):
    if mm_mode is None:
        mm_mode = MM_MODE
    if mm_mode not in _NC_CACHE:
        _NC_CACHE[mm_mode] = build_program(mm_mode)
    return _NC_CACHE[mm_mode]


def make_host_inputs(in_features, qkv_proj_weight, o_proj_weight,
                     mm_mode=None, ncores=NCORES):
    """Build the per-core input maps (the TP shard step)."""
    x = np.asarray(in_features, dtype=np.float32)
    qkv = np.asarray(qkv_proj_weight, dtype=np.float32)
    wo = np.asarray(o_proj_weight, dtype=np.float32)
    b, s, d = x.shape
    if mm_mode is None:
        mm_mode = MM_MODE
    np_st = ml_dtypes.bfloat16 if mm_mode == "bf16" else np.float32
    np_vt = ml_dtypes.bfloat16 if mm_mode == "bf16" else np.float32

    # pre-swizzled fully-contiguous DMA layouts (see build_program):
    # xT[g, p, c, n] = x[g*512+n, c*128+p]
    qt_, t_all = 512, b * s
    n_groups, n_kd = t_all // qt_, d // 128
    xTg = np.ascontiguousarray(
        x.reshape(n_groups, qt_, n_kd, 128).transpose(0, 3, 2, 1)
    ).astype(np_st)
    kk = np.arange(128)[:, None]
    qq = np.arange(128)[None, :]
    dmask = (qq >= kk).astype(np_st)
    ident = np.eye(128, dtype=np_vt)

    in_maps = []
    for c in range(ncores):
        rows = slice(c * DLOC, (c + 1) * DLOC)
        wstack = np.concatenate([qkv[i][rows, :] for i in range(3)], axis=0)
        # wqkvT[p, c, n] = wstack[n, c*128+p]
        wqkvT = np.ascontiguousarray(
            wstack.T.reshape(n_kd, 128, 3 * DLOC).transpose(1, 0, 2)
        ).astype(np_st)
        woT = np.ascontiguousarray(wo[:, rows].T).astype(np_st)
        in_maps.append({"xT": xTg, "wqkvT": wqkvT, "woT": woT,
                        "dmask": dmask, "ident": ident})
    return in_maps


def kernel(**inputs):
    nh = inputs.get("num_heads", NH)
    nh = int(np.asarray(nh)) if not isinstance(nh, int) else nh
    assert nh == NH, f"kernel hardcoded for {NH} heads, got {nh}"

    nc = _get_program()
    in_maps = make_host_inputs(inputs["in_features"],
                               inputs["qkv_proj_weight"],
                               inputs["o_proj_weight"])
    res = run_bass_kernel_spmd(nc, in_maps, list(range(NCORES)))
    acc = np.zeros((T, D), dtype=np.float64)
    for c in range(NCORES):
        acc += np.asarray(res.results[c]["part"], dtype=np.float64)
    return acc.reshape(B, S, D).astype(np.float32)
